# revision 22
# baseline (speedup 1.0000x reference)
"""Trainium2 Bass kernel for a single attention head.

Problem: X[4,4096,1024], Wq/Wk/Wv[1024,128] ->
  softmax((X@Wq)(X@Wk)^T / sqrt(1024)) @ (X@Wv)   -> [4,4096,128]

Sharding: 8 cores = 4 batches x 2 query-halves. Each core receives the full
X of its batch (rolled so its query half is rows [0:2048)), computes K/V for
all 4096 keys and flash-style attention for its 2048 queries.

On-core algorithm (all matmuls bf16 inputs, fp32 PSUM accumulation):
  1. X -> bf16 (cast DMA) -> X^T via XBAR transpose-DMA.
  2. K^T[h,n], V^T[h,n], Q^T[h,q] projections; V^T -> V[k,h] via transpose-DMA.
  3. Transposed flash attention per 1024-query chunk:
       S^T[k,q] = K_tile @ Q^T   (PSUM)
       P^T = exp(S^T/32)         (ACT, bf16 out)
       O^T[h,q] += V_tile^T @ P^T  ;  l[1,q] += ones^T @ P^T
     Epilogue: PE-transpose O^T and l, scale by 1/l, DMA out.
"""

import numpy as np

B, N, D, H = 4, 4096, 1024, 128
NCORES = 8
QSPLIT = 2  # cores per batch (query halves)
NQ = N // QSPLIT
SCALE = 1.0 / float(np.sqrt(np.float32(D)))
P = 128  # partitions
FB = 512  # matmul free-dim block (one fp32 PSUM bank)


def emit_attention(tc, X, Wq, Wk, Wv, O, n=N, d=D, nq=NQ, qc=1024):
    """Emit the single-core attention program into TileContext tc.

    X: [n, d] f32 DRAM (queries are rows [0:nq)); W*: [d, H] f32; O: [nq, H] f32.
    """
    import concourse.mybir as mybir
    from concourse.masks import make_identity

    nc = tc.nc
    dt = mybir.dt
    f32, bf16 = dt.float32, dt.bfloat16
    AF = mybir.ActivationFunctionType

    DT = d // P   # d tiles (contraction tiles for projections)
    NT = n // P   # key tiles
    qc = min(qc, nq)
    QB = qc // P  # 128-query blocks per chunk
    assert nq % qc == 0 and d % P == 0 and n % P == 0 and qc % P == 0

    from contextlib import ExitStack

    with ExitStack() as ctx:
        cpool = ctx.enter_context(tc.tile_pool(name="const", bufs=1))
        big = ctx.enter_context(tc.tile_pool(name="big", bufs=1))
        # attention-phase SBUF pools are allocated up front so their
        # addresses never overlap the released X staging pool (released-zone
        # deps would pile waits onto the output DMAs past the HW limit)
        ptp = ctx.enter_context(tc.tile_pool(name="pt", bufs=3))
        epp = ctx.enter_context(tc.tile_pool(name="ep", bufs=2))

        # Sacrificial first HWDGE DMA: the first SP-path DMA of a kernel
        # picks up a serialization wait against every outstanding SWDGE
        # lane; issue it before any SWDGE DMA exists so the real output
        # DMAs stay within the HW per-DMA wait limit.
        sac = cpool.tile([1, 16], f32)
        nc.sync.dma_start(sac[:], X[0:1, 0:16])

        ident = cpool.tile([P, P], f32)
        make_identity(nc, ident[:])
        ident_bf = cpool.tile([P, P], bf16)
        make_identity(nc, ident_bf[:])
        ones = cpool.tile([P, 1], bf16)
        nc.gpsimd.memset(ones[:], 1.0)

        w_sb = {}
        for name, w in (("wq", Wq), ("wk", Wk), ("wv", Wv)):
            t = cpool.tile([P, DT * H], bf16, tag=name)
            nc.gpsimd.dma_start(
                t[:].rearrange("p (t h) -> p t h", t=DT),
                w.rearrange("(t p) h -> p t h", p=P),
            )
            w_sb[name] = t

        xt = big.tile([P, DT * n], bf16)    # X^T: [d%128, dt*n + ncol]
        kT = big.tile([P, n], bf16)         # K^T[h, n]
        qT = big.tile([P, nq], bf16)        # Q^T[h, q]
        vT = big.tile([P, n], bf16)         # V^T[h, n] (staging)
        v_sb = big.tile([P, NT * H], bf16)  # V[k%128, kt*H + h]

        # ---- Phases 1+2: load X (f32->bf16 cast DMA), transpose to X^T via
        # PE (transpose-mode matmuls against identity), then projections.
        xt3 = xt[:].rearrange("p (t nn) -> p t nn", t=DT)
        LB = min(4, NT)  # n-row blocks per load DMA: few big DMAs, all into
        # one fully-resident staging tile -- slot recycling or per-block DMAs
        # would add WAW/ring-credit waits beyond the HW per-DMA wait limit.
        with (
            tc.tile_pool(name="xbfp", bufs=1) as xbf_pool,
            tc.tile_pool(name="p12", bufs=3, space="PSUM") as p12,
        ):
            xbf = xbf_pool.tile([P, NT * d], bf16)
            for nt0 in range(0, NT, LB):
                nc.gpsimd.dma_start(
                    xbf[:, nt0 * d:(nt0 + LB) * d].rearrange(
                        "p (a dd) -> p a dd", a=LB
                    ),
                    X[nt0 * P:(nt0 + LB) * P, :].rearrange(
                        "(a p) dd -> p a dd", p=P
                    ),
                )
                for a in range(LB):
                    nt = nt0 + a
                    xtp = p12.tile([P, DT * P], bf16, tag="xtp")
                    for t in range(DT):
                        nc.tensor.transpose(
                            xtp[:, t * P:(t + 1) * P],
                            xbf[:, nt * d + t * P: nt * d + (t + 1) * P],
                            ident_bf[:],
                        )
                    src = xtp[:].rearrange("p (t c) -> p t c", t=DT)
                    dst = xt3[:, :, nt * P:(nt + 1) * P]
                    if nt % 2:
                        nc.scalar.copy(dst, src)
                    else:
                        nc.vector.tensor_copy(dst, src)

            def project(wname, dst, ncols):
                for c0 in range(0, ncols, FB):
                    w = min(FB, ncols - c0)
                    ps = p12.tile([P, FB], f32, tag="pps")
                    for t in range(DT):
                        nc.tensor.matmul(
                            ps[:, :w],
                            w_sb[wname][:, t * H:(t + 1) * H],
                            xt[:, t * n + c0: t * n + c0 + w],
                            start=(t == 0),
                            stop=(t == DT - 1),
                        )
                    nc.vector.tensor_copy(dst[:, c0:c0 + w], ps[:, :w])

            project("wk", kT, n)
            project("wv", vT, n)
            project("wq", qT, nq)

            # V^T -> V via PE transposes
            for kt in range(NT):
                vp = p12.tile([P, P], bf16, tag="xtp")
                nc.tensor.transpose(
                    vp[:], vT[:, kt * P:(kt + 1) * P], ident_bf[:]
                )
                nc.vector.tensor_copy(v_sb[:, kt * H:(kt + 1) * H], vp[:])

        # ---- Phase 3: attention ----
        with ExitStack() as actx:
            stp = actx.enter_context(tc.tile_pool(name="stps", bufs=2, space="PSUM"))
            accp = actx.enter_context(tc.tile_pool(name="accps", bufs=1, space="PSUM"))

            for q0 in range(0, nq, qc):
                out_ps = accp.tile([P, qc], f32, tag="out")
                l_ps = accp.tile([1, qc], f32, tag="l")
                for kt in range(NT):
                    st = stp.tile([P, qc], f32, tag="st")
                    for j in range(0, qc, FB):
                        w = min(FB, qc - j)
                        nc.tensor.matmul(
                            st[:, j:j + w],
                            kT[:, kt * P:(kt + 1) * P],
                            qT[:, q0 + j: q0 + j + w],
                            start=True, stop=True,
                        )
                    pT = ptp.tile([P, qc], bf16, tag="pt")
                    nc.scalar.activation(pT[:], st[:], AF.Exp, scale=SCALE)
                    for j in range(0, qc, FB):
                        w = min(FB, qc - j)
                        nc.tensor.matmul(
                            out_ps[:, j:j + w],
                            v_sb[:, kt * H:(kt + 1) * H],
                            pT[:, j:j + w],
                            start=(kt == 0), stop=(kt == NT - 1),
                        )
                        nc.tensor.matmul(
                            l_ps[:, j:j + w],
                            ones[:],
                            pT[:, j:j + w],
                            start=(kt == 0), stop=(kt == NT - 1),
                        )

                # epilogue: 1/l, transpose O^T -> O, scale, store
                l_sb = epp.tile([1, qc], f32, tag="lsb")
                nc.vector.tensor_copy(l_sb[:], l_ps[:])
                r_sb = epp.tile([P, QB], f32, tag="rsb")
                for blk in range(QB):
                    lt = stp.tile([P, 1], f32, tag="st")
                    nc.tensor.transpose(
                        lt[:], l_sb[:, blk * P:(blk + 1) * P], ident[:1, :1]
                    )
                    nc.vector.reciprocal(r_sb[:, blk:blk + 1], lt[:])
                ob = epp.tile([P, qc], f32, tag="ob")
                nc.vector.tensor_copy(ob[:], out_ps[:])
                o_sb = epp.tile([P, QB * H], f32, tag="osb")
                for blk in range(QB):
                    ot = stp.tile([P, P], f32, tag="st")
                    nc.tensor.transpose(ot[:], ob[:, blk * P:(blk + 1) * P], ident[:])
                    nc.scalar.mul(
                        o_sb[:, blk * H:(blk + 1) * H], ot[:], r_sb[:, blk:blk + 1]
                    )
                nc.sync.dma_start(
                    O[q0:q0 + qc, :].rearrange("(qb p) h -> p qb h", p=P),
                    o_sb[:].rearrange("p (qb h) -> p qb h", qb=QB),
                )


def build_bass(n=N, d=D, nq=NQ, qc=1024):
    import concourse.mybir as mybir
    from concourse import bacc
    from concourse.tile import TileContext

    dt = mybir.dt
    nc = bacc.Bacc("TRN2", target_bir_lowering=False, debug=False)
    X = nc.dram_tensor("X", [n, d], dt.float32, kind="ExternalInput").ap()
    Wq = nc.dram_tensor("Wq", [d, H], dt.float32, kind="ExternalInput").ap()
    Wk = nc.dram_tensor("Wk", [d, H], dt.float32, kind="ExternalInput").ap()
    Wv = nc.dram_tensor("Wv", [d, H], dt.float32, kind="ExternalInput").ap()
    O = nc.dram_tensor("O", [nq, H], dt.float32, kind="ExternalOutput").ap()

    with TileContext(nc) as tc:
        emit_attention(tc, X, Wq, Wk, Wv, O, n=n, d=d, nq=nq, qc=qc)
    nc.compile()  # bacc passes: split multi-waits into EVSEM chains, etc.
    return nc


_CACHED = {}


def _get_nc():
    if "nc" not in _CACHED:
        _CACHED["nc"] = build_bass()
    return _CACHED["nc"]


def kernel(X, Wq, Wk, Wv, trace=False):
    """Full-input entry point: X [4,4096,1024] f32 -> [4,4096,128] f32."""
    from concourse.bass_utils import run_bass_kernel_spmd

    X = np.ascontiguousarray(X, dtype=np.float32)
    Wq = np.ascontiguousarray(Wq, dtype=np.float32)
    Wk = np.ascontiguousarray(Wk, dtype=np.float32)
    Wv = np.ascontiguousarray(Wv, dtype=np.float32)

    nc = _get_nc()
    in_maps = []
    for core in range(NCORES):
        b, half = core // QSPLIT, core % QSPLIT
        xb = X[b]
        if half:
            # roll so this core's queries are rows [0:NQ); key set is unchanged
            xb = np.concatenate([xb[NQ:], xb[:NQ]], axis=0)
        in_maps.append({"X": xb, "Wq": Wq, "Wk": Wk, "Wv": Wv})

    res = run_bass_kernel_spmd(
        nc, in_maps, core_ids=list(range(NCORES)), trace=trace
    )
    out = np.empty((B, N, H), dtype=np.float32)
    for core in range(NCORES):
        b, half = core // QSPLIT, core % QSPLIT
        out[b, half * NQ:(half + 1) * NQ] = res.results[core]["O"]
    if trace:
        return out, res
    return out
